# revision 7
# baseline (speedup 1.0000x reference)
"""Trainium2 Bass kernel for FlowNet-C CorrelationCost.

Problem: out[b,i,j, tj*21+ti] = (1/C) * sum_c A[b,i,j,c] * Bz[b, i+dy, j+dx, c]
with dy = 2*tj - 20, dx = 2*ti - 20, Bz = B zero-padded by 20 spatially.
Shapes: A, B = [16, 48, 64, 256] f32 -> out [16, 48, 64, 441] f32.

Strategy
--------
- Pure data-parallel: batch 16 -> 2 images per NeuronCore (8 cores, SPMD).
- Host pre-transposes inputs to channel-major [b, C, H, W] so DMA loads land
  in SBUF as [C-chunk(128) partitions, spatial] directly (no on-chip transpose).
- PE formulation: contract over C. For an i-pack {i0, i0+2, i0+4, i0+6} (same
  parity) and a column-parity class p, the stationary operand is
  A[c, pack x 32 same-parity columns] (128x128) and the moving operand streams
  B[c, r x 32 same-parity columns] for all B rows r with |r - i| <= 20 for some
  i in the pack. PSUM[m=(i,j), n=(r,jj)] then holds every correlation product
  with dy = r - i, dx = jj - j (dx even => j, jj same parity => parity split).
- fp32 data uses the float32r PE fast path (1 cycle/column at N >= 256).
- PSUM -> SBUF copy applies the 1/C scale (alternating ScalarE/VectorE), then
  one DMA per chunk ships the whole [128, ncols] block to DRAM. The host
  extracts the valid diagonal band (numpy as_strided) and assembles the output.

The harness calls kernel(**inputs) with the FULL inputs; this file is
self-contained (shapes hardcoded).
"""

import os
from contextlib import ExitStack

import numpy as np

import concourse.bass as bass
import concourse.tile as tile
from concourse import bacc, mybir

B_FULL, H, W, C = 16, 48, 64, 256
N_CORES = 8
B_PER = B_FULL // N_CORES  # batches per core
MD = 20                    # max displacement
D = 21                     # displacements per axis
PACK = 4                   # i rows packed into one stationary operand
NCOLS_MAX = 512            # one fp32 PSUM bank
F32 = mybir.dt.float32
F32R = mybir.dt.float32r   # fp32 PE fast path (1 cyc/col at N>=256)

USE_F32R = os.environ.get("CORR_NO_F32R", "") == ""


def plan_groups():
    """(pack, r_list) per i-pack: pack = 4 same-parity rows, r_list = B rows
    (same parity, step 2) needed by any row in the pack."""
    groups = []
    for par in (0, 1):
        i_vals = list(range(par, H, 2))
        for k in range(0, len(i_vals), PACK):
            pack = i_vals[k:k + PACK]
            r_lo = max(0, pack[0] - MD)
            r_hi = min(H - 1, pack[-1] + MD)
            r_list = [r for r in range(r_lo, r_hi + 1) if (r - pack[0]) % 2 == 0]
            groups.append((pack, r_list))
    return groups


def chunk_rs(r_list):
    """Split the r list into chunks of <= 16 rows (<= 512 cols, one PSUM bank),
    keeping every chunk >= 8 rows (256 cols) for the f32r full-rate path."""
    n = len(r_list)
    if n <= 16:
        return [r_list]
    h = (n + 1) // 2
    return [r_list[:h], r_list[h:]]


GROUPS = plan_groups()
N_GROUPS = len(GROUPS)            # 12 i-packs
MAX_CHUNKS = max(len(chunk_rs(r)) for _, r in GROUPS)  # 2


def pack_inputs(a_t, b_t):
    """Channel-major [b, C, H, W] -> matmul-ready packed layouts.

    a_packed[b, c, par, pk, p, k, j32] = a_t[b, c, 8*pk + 2*k + par, 2*j32 + p]
    b_packed[b, c, p, par, r2, jj32]  = b_t[b, c, 2*r2 + par, 2*jj32 + p]

    so that lhsT = a[:, par, pk, p, :] and rhs = b[:, p, par, r2 slice, :] are
    single-free-dim contiguous APs (a BIR matmul requirement).
    """
    nb = a_t.shape[0]
    ap = a_t.reshape(nb, C, 6, PACK, 2, 32, 2).transpose(0, 1, 4, 2, 6, 3, 5)
    bp = b_t.reshape(nb, C, 24, 2, 32, 2).transpose(0, 1, 5, 3, 2, 4)
    return (np.ascontiguousarray(ap).reshape(nb, C, 2, 6, 2, PACK * 32),
            np.ascontiguousarray(bp).reshape(nb, C, 2, 2, 24 * 32))


def build_program():
    nc = bacc.Bacc("TRN2", target_bir_lowering=False, debug=False)

    mm_dt = F32R if USE_F32R else F32
    a_d = nc.dram_tensor("a_t", [B_PER, C, 2, 6, 2, PACK * 32], mm_dt,
                         kind="ExternalInput")
    b_d = nc.dram_tensor("b_t", [B_PER, C, 2, 2, 24 * 32], mm_dt,
                         kind="ExternalInput")
    # raw matmul blocks: [b, group, parity, chunk, 128, 512]
    o_d = nc.dram_tensor(
        "out_raw", [B_PER, N_GROUPS, 2, MAX_CHUNKS, 128, NCOLS_MAX], F32,
        kind="ExternalOutput",
    )

    with tile.TileContext(nc) as tc, ExitStack() as ctx:
        inp = ctx.enter_context(tc.tile_pool(name="inp", bufs=1))
        psum = ctx.enter_context(
            tc.tile_pool(name="psum", bufs=8, space=bass.MemorySpace.PSUM))
        stage = ctx.enter_context(tc.tile_pool(name="stage", bufs=8))

        # Load both batches of both tensors fully into SBUF (packed layout).
        a_sb = {}
        b_sb = {}
        for b in range(B_PER):
            for cc in range(C // 128):
                ta = inp.tile([128, 2, 6, 2, PACK * 32], mm_dt, tag=f"a{b}_{cc}")
                nc.sync.dma_start(ta[:], a_d[b, cc * 128:(cc + 1) * 128])
                a_sb[b, cc] = ta
                tb = inp.tile([128, 2, 2, 24 * 32], mm_dt, tag=f"b{b}_{cc}")
                nc.sync.dma_start(tb[:], b_d[b, cc * 128:(cc + 1) * 128])
                b_sb[b, cc] = tb

        flip = 0
        for b in range(B_PER):
            for gi, (pack, r_list) in enumerate(GROUPS):
                par = pack[0] % 2
                pk = (pack[0] // 2) // PACK
                for p in (0, 1):
                    # stationary: [128c, 128 = 4i x 32j]
                    lhs = [
                        a_sb[b, cc][:, par, pk, p, :]
                        for cc in range(C // 128)
                    ]
                    for ci, rs in enumerate(chunk_rs(r_list)):
                        r2lo, nr = rs[0] // 2, len(rs)
                        ncols = nr * 32
                        ps = psum.tile([128, NCOLS_MAX], F32, tag="ps")
                        for cc in range(C // 128):
                            rhs = b_sb[b, cc][
                                :, p, par, r2lo * 32:(r2lo + nr) * 32]
                            nc.tensor.matmul(
                                ps[:, :ncols], lhs[cc], rhs,
                                start=(cc == 0), stop=(cc == C // 128 - 1),
                            )
                        st = stage.tile([128, NCOLS_MAX], F32, tag="st")
                        if flip:
                            nc.scalar.mul(st[:, :ncols], ps[:, :ncols], 1.0 / C)
                        else:
                            nc.vector.tensor_scalar_mul(
                                st[:, :ncols], ps[:, :ncols], 1.0 / C)
                        flip ^= 1
                        nc.sync.dma_start(
                            o_d[b, gi, p, ci, :, :ncols], st[:, :ncols])

    nc.compile()
    return nc


_NC_CACHE = None


def _get_program():
    global _NC_CACHE
    if _NC_CACHE is None:
        _NC_CACHE = build_program()
    return _NC_CACHE


def assemble_output(raw_all):
    """raw_all: [nb, N_GROUPS, 2, MAX_CHUNKS, 128, 512] f32 (already scaled)
    -> out [nb, H, W, D*D] f32."""
    nb = raw_all.shape[0]
    # band tensor: [nb, H, 2(p), 32(j32), D(dy), 32(jj32)]
    band = np.zeros((nb, H, 2, 32, D, 32), np.float32)
    for gi, (pack, r_list) in enumerate(GROUPS):
        for ci, rs in enumerate(chunk_rs(r_list)):
            nr = len(rs)
            # [B, 2p, 128, nr*32] -> [B, 2p, 4i, 32j, nr, 32jj]
            blk = raw_all[:, gi, :, ci, :, :nr * 32].reshape(
                nb, 2, PACK, 32, nr, 32)
            for k, i in enumerate(pack):
                for ridx, r in enumerate(rs):
                    dy = r - i
                    if abs(dy) > MD:
                        continue
                    dyi = (dy + MD) // 2
                    # [B, 2p, 32j, 32jj] -> band[:, i, p, j32, dyi, jj32]
                    band[:, i, :, :, dyi, :] = blk[:, :, k, :, ridx, :]
    out = np.zeros((nb, H, W, D, D), np.float32)
    s = band.strides
    for p in (0, 1):
        for ti in range(D):
            delta = ti - MD // 2  # dx/2
            j32_lo = max(0, -delta)
            j32_hi = min(32, 32 - delta)
            n = j32_hi - j32_lo
            if n <= 0:
                continue
            v = np.lib.stride_tricks.as_strided(
                band[:, :, p, j32_lo:, :, j32_lo + delta:],
                shape=(nb, H, n, D),
                strides=(s[0], s[1], s[3] + s[5], s[4]),
            )
            out[:, :, 2 * np.arange(j32_lo, j32_hi) + p, :, ti] = \
                v.transpose(2, 0, 1, 3)
    return out.reshape(nb, H, W, D * D)


def kernel(input_a: np.ndarray, input_b: np.ndarray) -> np.ndarray:
    from concourse.bass_utils import run_bass_kernel_spmd

    a = np.asarray(input_a, np.float32).transpose(0, 3, 1, 2)  # [B, C, H, W]
    bt = np.asarray(input_b, np.float32).transpose(0, 3, 1, 2)
    a, bt = pack_inputs(a, bt)

    nc = _get_program()
    core_ids = list(range(N_CORES))
    in_maps = [
        {"a_t": a[c * B_PER:(c + 1) * B_PER], "b_t": bt[c * B_PER:(c + 1) * B_PER]}
        for c in core_ids
    ]
    res = run_bass_kernel_spmd(nc, in_maps, core_ids)
    raw_all = np.concatenate(
        [res.results[c]["out_raw"] for c in core_ids], axis=0)
    return assemble_output(raw_all)
